# revision 22
# baseline (speedup 1.0000x reference)
"""GAT layer (PyG GATConv eval, 2 heads x 128, self-loops, ELU) on 8 trn2 cores.

Strategy (dst-sharded, per core):
  Phase A: per 128-node tile of full h: PE-transpose -> matmul with Wa4
           (=W.att contractions) -> write a_src to DRAM table TA[N,64] (256B rows).
  Phase A-bis: same on this core's dst shard -> a_dst resident in SBUF.
  Phase B: edges grouped by dst block (128 dsts), chunks of 128 edges.
           dma_gather of h rows (512B) + TA rows (256B) by src id (int16
           lo/hi table split).  Per chunk: dst one-hot masks via iota+is_equal,
           ex = exp(leakyrelu(a_src+a_dst)), GT_h[k,d] += (ex_h*Hg)^T M (PE),
           segsum[d,h] += M^T ex (PE).  Per block: U_h = (GT_h)^T W_h via PE,
           normalize by segsum, +bias, ELU, write out rows.
Softmax max-subtraction is skipped (shift-invariant; logits are O(10) so fp32
exp cannot overflow).
"""
import math
from contextlib import ExitStack

import numpy as np

HEADS = 2
C = 128
IN = 128
N = 50000
NC_CORES = 8
SH = N // NC_CORES            # 6250 dst nodes per core
NBLK = math.ceil(SH / 128)    # 49 dst blocks per core
SHP = NBLK * 128              # padded shard rows 6272
NTILE = math.ceil(N / 128)    # 391 tiles of full h
LO = 32768                    # int16 gather index split
GRP = 4                       # dst blocks per dma_gather call group
NEG_SLOPE = 0.2

_CACHE = {}


# ----------------------------------------------------------------- host prep
def _prep(edge_index):
    src = np.concatenate([edge_index[0], np.arange(N, dtype=np.int64)])
    dst = np.concatenate([edge_index[1], np.arange(N, dtype=np.int64)])
    src = src.astype(np.int64)
    core = dst // SH
    blk = (dst % SH) // 128
    dloc = (dst % SH) % 128
    half = (src >= LO).astype(np.int64)          # 0 = lo, 1 = hi

    key = (core * NBLK + blk) * 2 + half
    order = np.argsort(key, kind="stable")
    key_s = key[order]
    src_s = src[order]
    dloc_s = dloc[order]

    ngroups = NC_CORES * NBLK * 2
    sizes = np.bincount(key_s, minlength=ngroups)
    starts = np.concatenate([[0], np.cumsum(sizes)[:-1]])
    rank = np.arange(len(key_s)) - starts[key_s]

    lo_sizes = sizes.reshape(NC_CORES, NBLK, 2)[:, :, 0]
    hi_sizes = sizes.reshape(NC_CORES, NBLK, 2)[:, :, 1]
    K_LO = int(math.ceil(lo_sizes.max() / 128))
    K_HI = int(math.ceil(hi_sizes.max() / 128))
    K = K_LO + K_HI

    half_s = key_s % 2
    slot = rank + half_s * (K_LO * 128)          # slot within block [0, K*128)

    core_s = key_s // (2 * NBLK)
    blk_s = (key_s // 2) % NBLK

    # int16 gather index per slot (dummy 0 for padding), dst_local (999 pad)
    idx16 = np.zeros((NC_CORES, NBLK, K * 128), dtype=np.int16)
    dstl = np.full((NC_CORES, NBLK, K * 128), 999.0, dtype=np.float32)
    idxv = np.where(half_s == 0, src_s, src_s - LO).astype(np.int16)
    idx16[core_s, blk_s, slot] = idxv
    dstl[core_s, blk_s, slot] = dloc_s.astype(np.float32)

    # wrapped-16 gather index layout per block: w[b, p, col] = idx[col*16 + p%16]
    sl = idx16.reshape(NC_CORES, NBLK, K * 8, 16)       # [., ., col, p16]
    w_lo = np.ascontiguousarray(
        np.broadcast_to(
            sl[:, :, : K_LO * 8].transpose(0, 1, 3, 2)[:, :, None, :, :],
            (NC_CORES, NBLK, 8, 16, K_LO * 8),
        ).reshape(NC_CORES, NBLK, 128, K_LO * 8)
    )
    w_hi = np.ascontiguousarray(
        np.broadcast_to(
            sl[:, :, K_LO * 8 :].transpose(0, 1, 3, 2)[:, :, None, :, :],
            (NC_CORES, NBLK, 8, 16, K_HI * 8),
        ).reshape(NC_CORES, NBLK, 128, K_HI * 8)
    )

    # dstl layouts: [b, p, j] (slot s = j*128+p); uint16 variant for the
    # per-block a_dst table lookup (pads point at entry 127)
    d3 = dstl.reshape(NC_CORES, NBLK, K, 128)           # [., ., j, p]
    dstl_pj = np.ascontiguousarray(d3.transpose(0, 1, 3, 2))  # [., ., p, j]
    dstl_jp = np.ascontiguousarray(d3)                  # [., ., j, p]
    return K_LO, K_HI, w_lo, w_hi, dstl_pj, dstl_jp


# ------------------------------------------------------------ device program
def _build(K_LO, K_HI, phase="full"):
    import concourse.bacc as bacc
    import concourse.bass as bass
    import concourse.mybir as mybir
    import concourse.tile as tile
    from concourse.masks import make_identity

    dt = mybir.dt
    op = mybir.AluOpType
    act = mybir.ActivationFunctionType
    K = K_LO + K_HI
    P = 128

    nc = bacc.Bacc("TRN2", target_bir_lowering=False, debug=False,
                   num_devices=NC_CORES)
    h = nc.dram_tensor("h", [N, IN], dt.float32, kind="ExternalInput")
    h_sh = nc.dram_tensor("h_sh", [SHP, IN], dt.float32, kind="ExternalInput")
    w_in = nc.dram_tensor("w_in", [IN, HEADS * C], dt.float32, kind="ExternalInput")
    asrc_in = nc.dram_tensor("asrc_in", [HEADS, C], dt.float32, kind="ExternalInput")
    adst_in = nc.dram_tensor("adst_in", [HEADS, C], dt.float32, kind="ExternalInput")
    bias_in = nc.dram_tensor("bias_in", [1, HEADS * C], dt.float32, kind="ExternalInput")
    wlo_in = nc.dram_tensor("wlo", [NBLK, P, K_LO * 8], dt.int16, kind="ExternalInput")
    whi_in = nc.dram_tensor("whi", [NBLK, P, K_HI * 8], dt.int16, kind="ExternalInput")
    dpj_in = nc.dram_tensor("dpj", [NBLK * P, K], dt.float32, kind="ExternalInput")
    djp_in = nc.dram_tensor("djp", [NBLK, K * P], dt.float32, kind="ExternalInput")
    ta = nc.dram_tensor("ta", [NTILE * P, 64], dt.float32)
    out_t = nc.dram_tensor("out", [SHP, HEADS * C], dt.float32, kind="ExternalOutput")

    hap = h.ap()
    STAGE = 17  # phase-A tiles staged per TA write burst (391 = 23*17)

    with tile.TileContext(nc) as tc, ExitStack() as ctx:
        const = ctx.enter_context(tc.tile_pool(name="const", bufs=1))
        ctxA = ctx.enter_context(ExitStack())
        sbA = ctxA.enter_context(tc.tile_pool(name="sbA", bufs=3))
        stg = ctxA.enter_context(tc.tile_pool(name="stg", bufs=2))

        # ---- constants
        ident = const.tile([P, P], dt.float32)
        make_identity(nc, ident[:])
        iota_row = const.tile([P, P], dt.float32)
        nc.gpsimd.iota(iota_row[:], pattern=[[1, P]], base=0, channel_multiplier=0,
                       allow_small_or_imprecise_dtypes=True)
        iota_col4 = const.tile([P, 512], dt.float32)
        nc.gpsimd.iota(iota_col4[:], pattern=[[0, 512]], base=0, channel_multiplier=1,
                       allow_small_or_imprecise_dtypes=True)
        w_sb = const.tile([P, HEADS * C], dt.float32)
        nc.sync.dma_start(w_sb[:], w_in.ap()[:, :])

        ones_col = const.tile([P, 1], dt.float32)
        nc.gpsimd.memset(ones_col[:], 1.0)
        ones1 = const.tile([1, P], dt.float32)
        nc.gpsimd.memset(ones1[:], 1.0)
        bias_bc = const.tile([P, HEADS * C], dt.float32)
        nc.sync.dma_start(bias_bc[:], bass.AP(bias_in, 0, [[0, P], [1, HEADS * C]]))

        # Wa4[k, 0:2] = sum_c W[k, h*C+c]*att_src[h, c];  cols 2:4 for att_dst
        wa4 = const.tile([P, 4], dt.float32)
        tmp_pool = ctxA.enter_context(tc.tile_pool(name="watmp", bufs=2))
        for hd in range(HEADS):
            for j, attt in enumerate((asrc_in, adst_in)):
                abc = tmp_pool.tile([P, C], dt.float32, tag="abc")
                nc.sync.dma_start(abc[:], bass.AP(attt, hd * C, [[0, P], [1, C]]))
                t = tmp_pool.tile([P, C], dt.float32, tag="t")
                nc.vector.tensor_tensor(
                    out=t[:], in0=w_sb[:, hd * C:(hd + 1) * C],
                    in1=abc[:], op=op.mult)
                nc.vector.tensor_reduce(
                    out=wa4[:, 2 * j + hd:2 * j + hd + 1], in_=t[:],
                    axis=mybir.AxisListType.X, op=op.add)

        psA = ctxA.enter_context(tc.tile_pool(name="psA", bufs=2, space="PSUM"))
        psA2 = ctxA.enter_context(tc.tile_pool(name="psA2", bufs=2, space="PSUM"))

        # ---- phase A: a_src table for all N (+ phase A-bis shard a_dst)
        adst_sb = const.tile([P, NBLK, 2], dt.float32)

        def attn_tile(src_ap, nrows):
            """load [nrows,128] h rows -> return psum [128,4] a-values tile."""
            ht = sbA.tile([P, IN], dt.float32, tag="ht")
            nc.sync.dma_start(ht[:nrows, :], src_ap)
            tp = psA.tile([P, P], dt.float32, tag="tp", space="PSUM")
            nc.tensor.transpose(out=tp[:], in_=ht[:], identity=ident[:])
            hT = sbA.tile([P, P], dt.float32, tag="hT")
            nc.scalar.copy(out=hT[:], in_=tp[:])
            a4 = psA2.tile([P, 4], dt.float32, tag="a4", space="PSUM")
            nc.tensor.matmul(out=a4[:], lhsT=hT[:], rhs=wa4[:], start=True, stop=True)
            return a4

        for t0 in range(0, NTILE, STAGE):
            nst = min(STAGE, NTILE - t0)
            st = stg.tile([P, STAGE, 4], dt.float32, tag="st")
            for g in range(nst):
                ti = t0 + g
                nrows = min(P, N - ti * P)
                a4 = attn_tile(hap[ti * P:ti * P + nrows, :], nrows)
                nc.scalar.copy(out=st[:, g, :], in_=a4[:])
            # burst write to TA rows [t0*128, (t0+nst)*128), cols 0:4
            out_ap = bass.AP(ta, t0 * P * 64, [[64, P], [P * 64, nst], [1, 4]])
            nc.gpsimd.dma_start(out_ap, st[:, :nst, :])

        for b in range(NBLK):
            a4 = attn_tile(h_sh.ap()[b * P:(b + 1) * P, :], P)
            nc.scalar.copy(out=adst_sb[:, b, 0:2], in_=a4[:, 2:4])

        ctxA.close()  # free phase-A SBUF + PSUM pools before phase B

        # ---- phase B
        gh = ctx.enter_context(tc.tile_pool(name="gh", bufs=2))
        gt = ctx.enter_context(tc.tile_pool(name="gt", bufs=2))
        gi = ctx.enter_context(tc.tile_pool(name="gi", bufs=2))
        mk = ctx.enter_context(tc.tile_pool(name="mk", bufs=3))
        sm = ctx.enter_context(tc.tile_pool(name="sm", bufs=3))
        fin = ctx.enter_context(tc.tile_pool(name="fin", bufs=2))
        psGT = ctx.enter_context(tc.tile_pool(name="psGT", bufs=2, space="PSUM"))
        psSS = ctx.enter_context(tc.tile_pool(name="psSS", bufs=1, space="PSUM"))
        psAD = ctx.enter_context(tc.tile_pool(name="psAD", bufs=1, space="PSUM"))
        psB = ctx.enter_context(tc.tile_pool(name="psB", bufs=1, space="PSUM"))
        psU = ctx.enter_context(tc.tile_pool(name="psU", bufs=1, space="PSUM"))

        taap = ta.ap()
        blk_range = [] if phase == "A" else list(range(0, NBLK, GRP))
        for g0 in blk_range:
            ng = min(GRP, NBLK - g0)
            ilo = gi.tile([P, GRP * K_LO * 8], dt.int16, tag="ilo")
            nc.sync.dma_start(
                ilo[:, : ng * K_LO * 8],
                bass.AP(wlo_in, g0 * P * K_LO * 8,
                        [[K_LO * 8, P], [P * K_LO * 8, ng], [1, K_LO * 8]]))
            ihi = gi.tile([P, GRP * K_HI * 8], dt.int16, tag="ihi")
            nc.sync.dma_start(
                ihi[:, : ng * K_HI * 8],
                bass.AP(whi_in, g0 * P * K_HI * 8,
                        [[K_HI * 8, P], [P * K_HI * 8, ng], [1, K_HI * 8]]))

            hg_lo = gh.tile([P, GRP * K_LO, IN], dt.float32, tag="hglo")
            nc.gpsimd.dma_gather(
                out_ap=hg_lo[:, : ng * K_LO, :], in_ap=hap[0:LO, :],
                idxs_ap=ilo[:, : ng * K_LO * 8], num_idxs=ng * K_LO * P,
                num_idxs_reg=ng * K_LO * P, elem_size=IN, single_packet=False)
            hg_hi = gh.tile([P, GRP * K_HI, IN], dt.float32, tag="hghi")
            nc.gpsimd.dma_gather(
                out_ap=hg_hi[:, : ng * K_HI, :], in_ap=hap[LO:N, :],
                idxs_ap=ihi[:, : ng * K_HI * 8], num_idxs=ng * K_HI * P,
                num_idxs_reg=ng * K_HI * P, elem_size=IN, single_packet=False)
            ta_lo = gt.tile([P, GRP * K_LO, 64], dt.float32, tag="talo")
            nc.gpsimd.dma_gather(
                out_ap=ta_lo[:, : ng * K_LO, :], in_ap=taap[0:LO, :],
                idxs_ap=ilo[:, : ng * K_LO * 8], num_idxs=ng * K_LO * P,
                num_idxs_reg=ng * K_LO * P, elem_size=64, single_packet=False)
            ta_hi = gt.tile([P, GRP * K_HI, 64], dt.float32, tag="tahi")
            nc.gpsimd.dma_gather(
                out_ap=ta_hi[:, : ng * K_HI, :], in_ap=taap[LO:NTILE * P, :],
                idxs_ap=ihi[:, : ng * K_HI * 8], num_idxs=ng * K_HI * P,
                num_idxs_reg=ng * K_HI * P, elem_size=64, single_packet=False)

            if phase == "gather":
                ob0 = fin.tile([P, HEADS * C], dt.float32, tag="ob")
                nc.vector.tensor_copy(out=ob0[:, 0:IN], in_=hg_lo[:, 0, :])
                nc.vector.tensor_copy(out=ob0[:, IN:IN + 64], in_=ta_lo[:, 0, :])
                nc.vector.tensor_copy(out=ob0[:, IN + 64:IN + 128],
                                      in_=ta_hi[:, 0, :])
                nc.sync.dma_start(out_t.ap()[(g0 // GRP) * P:(g0 // GRP + 1) * P, :], ob0[:])
                continue
            for bg in range(ng):
                b = g0 + bg
                dpj = sm.tile([P, K], dt.float32, tag="dpj")
                nc.sync.dma_start(dpj[:], dpj_in.ap()[b * P:(b + 1) * P, :])
                djp = sm.tile([1, K * P], dt.float32, tag="djp")
                nc.sync.dma_start(djp[:], djp_in.ap()[b:b + 1, :])

                # a_dst per edge slot via transposed one-hot matmuls,
                # 4 chunks per broadcast round
                adp = psAD.tile([P, K, 2], dt.float32, tag="adp", space="PSUM")
                for j0 in range(0, K, 4):
                    nb = min(4, K - j0)
                    bc = psB.tile([P, 512], dt.float32, tag="bc", space="PSUM")
                    nc.tensor.matmul(out=bc[:, : nb * P], lhsT=ones1[:],
                                     rhs=djp[:, j0 * P:(j0 + nb) * P],
                                     start=True, stop=True)
                    mt4 = mk.tile([P, 512], dt.float32, tag="mt4")
                    nc.vector.tensor_tensor(out=mt4[:, : nb * P],
                                            in0=iota_col4[:, : nb * P],
                                            in1=bc[:, : nb * P], op=op.is_equal)
                    for jj in range(nb):
                        nc.tensor.matmul(out=adp[:, j0 + jj, :],
                                         lhsT=mt4[:, jj * P:(jj + 1) * P],
                                         rhs=adst_sb[:, b, :],
                                         start=True, stop=True)

                # logits -> ex for every slot of the block, batched wide ops
                tsum = sm.tile([P, K, 2], dt.float32, tag="tsum")
                nc.vector.tensor_tensor(
                    out=tsum[:, :K_LO, :],
                    in0=ta_lo[:, bg * K_LO:(bg + 1) * K_LO, 0:2],
                    in1=adp[:, :K_LO, :], op=op.add)
                nc.vector.tensor_tensor(
                    out=tsum[:, K_LO:, :],
                    in0=ta_hi[:, bg * K_HI:(bg + 1) * K_HI, 0:2],
                    in1=adp[:, K_LO:, :], op=op.add)
                u02 = sm.tile([P, K, 2], dt.float32, tag="u02")
                nc.vector.tensor_scalar(out=u02[:], in0=tsum[:], scalar1=NEG_SLOPE,
                                        scalar2=None, op0=op.mult)
                lr = sm.tile([P, K, 2], dt.float32, tag="lr")
                nc.vector.tensor_tensor(out=lr[:], in0=tsum[:], in1=u02[:],
                                        op=op.max)
                ex = sm.tile([P, K, 2], dt.float32, tag="ex")
                nc.scalar.activation(out=ex[:], in_=lr[:], func=act.Exp)

                gtt = psGT.tile([P, 2 * P], dt.float32, tag="gt", space="PSUM")
                ss0 = psSS.tile([P, 1], dt.float32, tag="ss0", space="PSUM")
                ss1 = psSS.tile([P, 1], dt.float32, tag="ss1", space="PSUM")

                for j in range(K):
                    if j < K_LO:
                        hgc = hg_lo[:, bg * K_LO + j, :]
                    else:
                        hgc = hg_hi[:, bg * K_HI + (j - K_LO), :]
                    st_ = j == 0
                    sp = j == K - 1
                    exm = mk.tile([P, 2 * P], dt.float32, tag="exm")
                    for hd, sstile in ((0, ss0), (1, ss1)):
                        nc.vector.tensor_scalar(
                            out=exm[:, hd * P:(hd + 1) * P], in0=iota_row[:],
                            scalar1=dpj[:, j:j + 1],
                            scalar2=ex[:, j, hd:hd + 1],
                            op0=op.is_equal, op1=op.mult)
                        nc.tensor.matmul(out=sstile[:],
                                         lhsT=exm[:, hd * P:(hd + 1) * P],
                                         rhs=ones_col[:], start=st_, stop=sp)
                    nc.tensor.matmul(out=gtt[:], lhsT=hgc, rhs=exm[:],
                                     start=st_, stop=sp)

                # ---- finalize block b
                rec = fin.tile([P, 2], dt.float32, tag="rec")
                nc.vector.reciprocal(out=rec[:, 0:1], in_=ss0[:])
                nc.vector.reciprocal(out=rec[:, 1:2], in_=ss1[:])
                ob = fin.tile([P, HEADS * C], dt.float32, tag="ob")
                for hd in range(HEADS):
                    gs = fin.tile([P, P], dt.float32, tag="gs")
                    nc.scalar.copy(out=gs[:], in_=gtt[:, hd * P:(hd + 1) * P])
                    u = psU.tile([P, P], dt.float32, tag="u", space="PSUM")
                    nc.tensor.matmul(out=u[:], lhsT=gs[:],
                                     rhs=w_sb[:, hd * C:(hd + 1) * C],
                                     start=True, stop=True)
                    o = fin.tile([P, C], dt.float32, tag="o")
                    nc.vector.tensor_scalar(
                        out=o[:], in0=u[:], scalar1=rec[:, hd:hd + 1],
                        scalar2=None, op0=op.mult)
                    o2 = fin.tile([P, C], dt.float32, tag="o2")
                    nc.vector.tensor_tensor(
                        out=o2[:], in0=o[:],
                        in1=bias_bc[:, hd * C:(hd + 1) * C], op=op.add)
                    a1 = fin.tile([P, C], dt.float32, tag="a1")
                    nc.vector.tensor_scalar(out=a1[:], in0=o2[:], scalar1=0.0,
                                            scalar2=None, op0=op.min)
                    e1 = fin.tile([P, C], dt.float32, tag="e1")
                    nc.scalar.activation(out=e1[:], in_=a1[:], func=act.Exp)
                    a3 = fin.tile([P, C], dt.float32, tag="a3")
                    nc.vector.tensor_scalar(out=a3[:], in0=o2[:], scalar1=0.0,
                                            scalar2=-1.0, op0=op.max, op1=op.add)
                    nc.vector.tensor_tensor(
                        out=ob[:, hd * C:(hd + 1) * C], in0=a3[:], in1=e1[:],
                        op=op.add)
                nc.sync.dma_start(out_t.ap()[b * P:(b + 1) * P, :], ob[:])

    nc.compile()
    return nc


def _get_program(K_LO, K_HI):
    key = (K_LO, K_HI)
    if key not in _CACHE:
        _CACHE[key] = _build(K_LO, K_HI)
    return _CACHE[key]


# ------------------------------------------------------------------- kernel
def kernel(h_node, edge_index, W, att_src, att_dst, bias):
    from concourse.bass_utils import run_bass_kernel_spmd

    h_node = np.asarray(h_node, dtype=np.float32)
    W = np.asarray(W, dtype=np.float32)
    att_src = np.asarray(att_src, dtype=np.float32)
    att_dst = np.asarray(att_dst, dtype=np.float32)
    bias = np.asarray(bias, dtype=np.float32).reshape(1, HEADS * C)

    K_LO, K_HI, w_lo, w_hi, dstl_pj, dstl_jp = _prep(np.asarray(edge_index))
    nc = _get_program(K_LO, K_HI)

    in_maps = []
    for c in range(NC_CORES):
        hs = np.zeros((SHP, IN), dtype=np.float32)
        hs[:SH] = h_node[c * SH:(c + 1) * SH]
        in_maps.append({
            "h": h_node, "h_sh": hs, "w_in": W, "asrc_in": att_src,
            "adst_in": att_dst, "bias_in": bias,
            "wlo": w_lo[c], "whi": w_hi[c],
            "dpj": dstl_pj[c].reshape(NBLK * 128, K_LO + K_HI),
            "djp": dstl_jp[c].reshape(NBLK, (K_LO + K_HI) * 128),
        })
    res = run_bass_kernel_spmd(nc, in_maps, core_ids=list(range(NC_CORES)))
    out = np.concatenate([res.results[c]["out"][:SH] for c in range(NC_CORES)], axis=0)
    return out
